# revision 29
# baseline (speedup 1.0000x reference)
"""Trainium2 Bass kernel for NetworksPlusCircuit.

Two MLPs (784->512->10, relu, softmax) over B=65536 samples each, then
P(sum=s) = sum_{a+c=s} p0[a]*p1[c]  -> [B, 19].

Sharding: pure data parallel over the batch across 8 NeuronCores.
Host-side prep: shard + transpose x to [784, B/8] per core and cast to
fp16 so the device DMA streams are halved and contiguous; weights
replicated (cast to fp16 on host).

Device-side per core (BC = 8192 samples per net):
  per round (512 samples of one net):
    - HWDGE loads xT tile [128f, 7fc, 1024b] fp16, one DMA per tile
      (each DMA costs ~650ns ring time); the first tile is split fine so
      round 0 starts right after the framework preamble
    - mm1: hT[j, b] += W1[f, j].T @ xT[f, b]   (PSUM, 4 j-chunks x 7 f-chunks,
      16-row tail as 4 concurrent row-tiled strips)
    - bias+relu -> SBUF fp16 (ACT)
    - mm2: l[b, 10]  += hT[j, b].T @ W2[j, 10] one matmul per mm1 slot, so
      each mm2 LDWEIGHTS (~95ns) hides in weight-port slack under an mm1
      stream (216ns) instead of 16 LDWs serializing
    - exp (ACT, PSUM -> SBUF block buffer), softmax denominator deferred
  per block (3 pair-rounds; final block is 1 round so its conv tail is
  short):
    - scale by exp(b2)/16, Z sums, digit-sum convolution via a skewed fp16
      write + diagonal-strided reduce (3 DVE ops instead of 21), normalize,
      DMA out
"""

import sys
import types

import numpy as np
import ml_dtypes
import concourse.bass as bass
import concourse.bacc as bacc
import concourse.mybir as mybir
import concourse.tile as tile
from concourse.bass_utils import run_bass_kernel_spmd

F32 = mybir.dt.float32
MM1DT = mybir.dt.float16    # mm1 dtype (x, W1): fp16 = same PE speed as bf16,
MM1NP = np.float16          # 8x less rounding error (measured equal exec time)
MM2DT = mybir.dt.float16    # mm2 dtype (ht, W2)
MM2NP = np.float16

NCORES = 8
B = 65536
BC = B // NCORES            # samples per core per net
F = 784                     # input features
HID = 512
NCLS = 10
NSUM = 19
FCH_FULL = F // 128         # 6 full feature chunks
FT = F - FCH_FULL * 128     # 16 tail features
FCH = FCH_FULL + 1          # 7 chunks
JCH = HID // 128            # 4 hidden chunks
# The 16 tail features are replicated at 4 32-row strips (rows 768+32k..+16)
# so the 4 per-j-chunk tail matmuls sit in distinct PE row-groups and run
# concurrently (row tiling).
FPAD = FCH * 128  # padded rows: 6*128 full chunks + 4 strips * 32 = 896
RN = 512                    # samples per compute round
XR = 1024                   # samples per x-load tile
XRR = XR // RN              # compute rounds per x tile
NR = BC // RN               # 16 pair-rounds per core
# conv blocks in pair-rounds: uneven so the last block's conv (the only
# one that can't overlap matmuls of a later round) is small
BLK_ROUNDS = [3, 3, 3, 3, 3, 1]
assert sum(BLK_ROUNDS) == NR
GPR = RN // 128             # 128-sample groups per pair-round


def pad_rows(a, dtype):
    """[F, ...] -> [FPAD, ...] with tail rows replicated at 4 32-row strips."""
    out = np.zeros((FPAD,) + a.shape[1:], dtype=dtype)
    out[: FCH_FULL * 128] = a[: FCH_FULL * 128]
    for k in range(JCH):
        base = FCH_FULL * 128 + 32 * k
        out[base : base + FT] = a[FCH_FULL * 128 :]
    return out


def build_nc():
    nc = bacc.Bacc("TRN2", target_bir_lowering=False, debug=False, num_devices=NCORES)

    xt = [
        nc.dram_tensor(f"xt{n}", [FPAD, BC], MM1DT, kind="ExternalInput")
        for n in range(2)
    ]
    # weights arrive host-prepacked in their on-chip layouts so every const
    # DMA is one contiguous line per partition (cheap descriptor generation)
    w1 = [
        nc.dram_tensor(f"w1_{n}", [128, FCH, HID], MM1DT, kind="ExternalInput")
        for n in range(2)
    ]
    w2 = [
        nc.dram_tensor(f"w2_{n}", [128, JCH, NCLS], MM2DT, kind="ExternalInput")
        for n in range(2)
    ]
    # cpack[p] = [b1_0 (4), b1_1 (4), eb2_0 (10), eb2_1 (10)]
    cpk = nc.dram_tensor("cpack", [128, 28], F32, kind="ExternalInput")
    out = nc.dram_tensor("out", [BC, NSUM], F32, kind="ExternalOutput")

    with tile.TileContext(nc) as tc:
        with (
            tc.tile_pool(name="consts", bufs=1) as consts,
            tc.tile_pool(name="xt", bufs=8) as xt_pool,
            tc.tile_pool(name="ht", bufs=3) as ht_pool,
            tc.tile_pool(name="eblk", bufs=2) as e_pool,
            tc.tile_pool(name="conv", bufs=1) as conv_pool,
            tc.tile_pool(name="outp", bufs=3) as out_pool,
            tc.tile_pool(name="hpsum", bufs=7, space="PSUM") as h_pool,
            tc.tile_pool(name="lpsum", bufs=1, space="PSUM") as l_pool,
        ):
            # ---- constants (HWDGE on the ACT ring so the x-tile loads own
            # the sync ring from t=0) ----
            w1_sb = []
            w2_sb = []
            for n in range(2):
                w = consts.tile([128, FCH, HID], MM1DT, name=f"w1sb{n}", tag=f"w1sb{n}")
                w1_sb.append(w)
            cpack = consts.tile([128, 28], F32, name="cpack", tag="cpack")

            def w1_load(n):
                # three DMAs in first-use order (tail chunk 6, then the full
                # chunks in halves) so round-0 matmuls start as pieces land
                for c0, c1 in ((FCH_FULL, FCH), (0, 3), (3, FCH_FULL)):
                    nc.scalar.dma_start(
                        out=w1_sb[n][:, c0:c1, :], in_=w1[n].ap()[:, c0:c1, :]
                    )

            w1_load(0)
            nc.scalar.dma_start(out=cpack, in_=cpk.ap())
            w1_load(1)
            for n in range(2):
                w2t = consts.tile([128, JCH, NCLS], MM2DT, name=f"w2sb{n}", tag=f"w2sb{n}")
                nc.scalar.dma_start(out=w2t, in_=w2[n].ap())
                w2_sb.append(w2t)
            b1_sb = [cpack[:, 0:JCH], cpack[:, JCH : 2 * JCH]]
            # eb2 for both nets as [128, 2, NCLS]
            eb2_bn = cpack[:, 2 * JCH : 2 * JCH + 2 * NCLS].rearrange(
                "p (n c) -> p n c", n=2
            )
            # skew buffer for the digit-sum convolution: per group, row a
            # (stride 21) holds p0[a]*p1[c] at col a+c+1, so the diagonal
            # read (stride-20 view) sums exactly the a+c==s terms; all other
            # cells stay zero from this one-time memset (rows never move).
            MAXG = max(BLK_ROUNDS) * GPR
            # fp16 products: eb2 is pre-scaled by 1/16 on the host so the
            # p0*p1 products stay in fp16 range; the z normalization divides
            # the scaling back out (scale-invariant)
            skew = consts.tile([128, MAXG, 216], MM2DT, name="skew", tag="skew")
            nc.vector.memset(skew, 0.0)

            # ---- conv over a range of groups ----
            def emit_conv(ev, gc, row0):
                # ev: e slice [128, 2, gc, NCLS]; writes out rows row0..
                nc.vector.tensor_tensor(
                    out=ev,
                    in0=ev,
                    in1=eb2_bn[:, :, None, :].to_broadcast([128, 2, gc, NCLS]),
                    op=mybir.AluOpType.mult,
                )
                z = conv_pool.tile([128, 2, gc], F32, name="z", tag="z")
                nc.vector.reduce_sum(out=z, in_=ev, axis=mybir.AxisListType.X)
                rz = conv_pool.tile([128, gc], F32, name="rz", tag="rz")
                nc.vector.tensor_tensor(
                    out=rz, in0=z[:, 0, :], in1=z[:, 1, :], op=mybir.AluOpType.mult
                )
                nc.vector.reciprocal(out=rz, in_=rz)
                # skewed write: (g, a, c) -> col a+c+1 of the stride-21 row a
                wview = skew[:, 0:gc, 1:211].rearrange(
                    "p g (a u) -> p g a u", a=NCLS
                )[:, :, :, 0:NCLS]
                nc.vector.tensor_tensor(
                    out=wview,
                    in0=ev[:, 1, :, None, :].to_broadcast([128, gc, NCLS, NCLS]),
                    in1=ev[:, 0, :, :, None].to_broadcast([128, gc, NCLS, NCLS]),
                    op=mybir.AluOpType.mult,
                )
                # diagonal read: (g, s, a) at 20a+s+1 sums the a+c==s products
                rview = skew[:, 0:gc, 1:201].rearrange(
                    "p g (a u) -> p g a u", a=NCLS
                )[:, :, :, 0:NSUM].transpose([0, 1, 3, 2])
                acc = out_pool.tile([128, gc, NSUM], F32, name="acc", tag="acc")
                nc.vector.reduce_sum(out=acc, in_=rview, axis=mybir.AxisListType.X)
                nc.vector.tensor_tensor(
                    out=acc,
                    in0=acc,
                    in1=rz[:, :, None].to_broadcast([128, gc, NSUM]),
                    op=mybir.AluOpType.mult,
                )
                nc.scalar.dma_start(
                    out=out.ap()[row0 : row0 + gc * 128, :].rearrange(
                        "(g p) s -> p g s", p=128
                    ),
                    in_=acc,
                )

            # mm2 + exp for a finished round, emitted ONE SMALL MATMUL AT A
            # TIME between the next round's mm1 matmuls: each mm2 LDWEIGHTS
            # (~95ns) hides in the weight-port slack under an mm1 stream
            # (216ns - its own 97ns LDW), instead of 16 LDWs serializing.
            pend = {"v": None, "k": 16, "pl": None}

            def mm2_step():
                if pend["v"] is None or pend["k"] >= 16:
                    return
                pn, pht, pe_t, prr, last, pgpb, prow0 = pend["v"]
                k = pend["k"]
                bc, jc = divmod(k, JCH)
                if k == 0:
                    pend["pl"] = l_pool.tile(
                        [128, RN // 128, NCLS], F32, name="pl", tag="pl"
                    )
                nc.tensor.matmul(
                    out=pend["pl"][:, bc, :],
                    lhsT=pht[:, jc, bc * 128 : (bc + 1) * 128],
                    rhs=w2_sb[pn][:, jc, :],
                    start=(jc == 0),
                    stop=(jc == JCH - 1),
                    skip_group_check=True,
                )
                pend["k"] = k + 1
                if pend["k"] == 16:
                    nc.scalar.activation(
                        out=pe_t[:, pn, prr * GPR : (prr + 1) * GPR, :],
                        in_=pend["pl"][:, :, :],
                        func=mybir.ActivationFunctionType.Exp,
                    )
                    if pn == 1 and last:
                        emit_conv(pe_t, pgpb, prow0)

            # ---- main loop (mm2/exp software-pipelined one round behind) ----
            xtiles = {}
            r = 0
            row0 = 0
            for blk, rpb in enumerate(BLK_ROUNDS):
                gpb = rpb * GPR
                # exp values for this block: [128, net, group, cls]
                e_t = e_pool.tile([128, 2, gpb, NCLS], F32, name=f"eb{rpb}", tag=f"eb{rpb}")
                for rr in range(rpb):
                    # x tiles hold XR samples (several compute rounds):
                    # bigger DMA lines (XR*2B per partition) + fewer DMAs.
                    # net0's chunks are queued before net1's so the first
                    # rounds pipeline; the first tile loads in column halves
                    # so the first round's matmuls start ~2.5us in.
                    if r % XRR == 0:
                        for n in range(2):
                            xtiles[n] = xt_pool.tile(
                                [128, FCH, XR], MM1DT, name="xt", tag="xt"
                            )
                        if r == 0:
                            # first tile in consumption order: net0's first
                            # round finely chunked (tail chunk 6 first) so the
                            # first matmuls start ~1.5us after the rings open;
                            # later pieces coarser (each DMA costs ~650ns of
                            # ring time)
                            splits = [
                                (n, 0, FCH, b0, b1)
                                for b0, b1 in ((0, RN), (RN, XR))
                                for n in range(2)
                            ]
                        else:
                            # steady state: one DMA per tile (~650ns of ring
                            # time per DMA instruction, so fewer is better)
                            splits = [(n, 0, FCH, 0, XR) for n in range(2)]
                        for n, c0, c1, b0, b1 in splits:
                            xsrc = xt[n].ap()[:, r * RN + b0 : r * RN + b1]
                            nc.sync.dma_start(
                                out=xtiles[n][:, c0:c1, b0:b1],
                                in_=xsrc[c0 * 128 : c1 * 128, :].rearrange(
                                    "(c p) b -> p c b", p=128
                                ),
                            )
                    for n in range(2):
                        xtile = xtiles[n][:, :, (r % XRR) * RN : (r % XRR + 1) * RN]

                        ph = [
                            h_pool.tile([128, RN], F32, name="ph", tag="ph")
                            for i in range(JCH)
                        ]
                        # tail features first: 4 concurrent row-tiled K=16
                        # matmuls (start=True initializes every psum element)
                        for jc in range(JCH):
                            p0 = 32 * jc
                            nc.tensor.matmul(
                                out=ph[jc][:, :],
                                lhsT=w1_sb[n][
                                    p0 : p0 + FT, FCH_FULL, jc * 128 : (jc + 1) * 128
                                ],
                                rhs=xtile[p0 : p0 + FT, FCH_FULL, :],
                                start=True,
                                stop=False,
                                tile_position=(p0, 0),
                            )
                        ht = ht_pool.tile([128, JCH, RN], MM2DT, name="ht", tag="ht")
                        # jc-major: each j-chunk finishes early so its relu
                        # (all on ACT) drains the psum slot while the round runs
                        for jc in range(JCH):
                            for fc in range(FCH_FULL):
                                nc.tensor.matmul(
                                    out=ph[jc][:, :],
                                    lhsT=w1_sb[n][:, fc, jc * 128 : (jc + 1) * 128],
                                    rhs=xtile[:, fc, :],
                                    start=False,
                                    stop=(fc == FCH_FULL - 1),
                                )
                                mm2_step()
                            nc.scalar.activation(
                                out=ht[:, jc, :],
                                in_=ph[jc][:, :],
                                func=mybir.ActivationFunctionType.Relu,
                                bias=b1_sb[n][:, jc : jc + 1],
                                scale=1.0,
                            )
                        pend["v"] = (n, ht, e_t, rr, rr == rpb - 1, gpb, row0)
                        pend["k"] = 0
                    r += 1
                row0 += gpb * 128
            while pend["k"] < 16:
                mm2_step()

    nc.compile()
    return nc


_NC_CACHE = {}


def _get_nc():
    if "nc" not in _NC_CACHE:
        _NC_CACHE["nc"] = build_nc()
    return _NC_CACHE["nc"]


def _install_ntff_hook():
    """Shim antenv.axon_hooks (absent in this image) so trace=True can work."""
    try:
        import antenv

        if hasattr(antenv, "axon_hooks"):
            return
        from trn_agent_boot.trn_boot import _ntff_profile_via_ctypes

        mod = types.ModuleType("antenv.axon_hooks")
        holder = {"hook": _ntff_profile_via_ctypes("/opt/axon/libaxon_pjrt.so")}
        mod.set_axon_ntff_profile_hook = lambda h: holder.__setitem__("hook", h)
        mod.get_axon_ntff_profile_hook = lambda: holder["hook"]
        sys.modules["antenv.axon_hooks"] = mod
        antenv.axon_hooks = mod
    except Exception:
        pass


def kernel(x, W1_0, b1_0, W2_0, b2_0, W1_1, b1_1, W2_1, b2_1, _trace=False):
    x = np.asarray(x, dtype=np.float32)

    xf = x.reshape(2, B, F)
    weights = {}
    cpack = np.zeros((128, 28), np.float32)
    for n, (W1n, b1n, W2n, b2n) in enumerate(
        [(W1_0, b1_0, W2_0, b2_0), (W1_1, b1_1, W2_1, b2_1)]
    ):
        w1p = pad_rows(np.asarray(W1n, np.float32).astype(MM1NP), MM1NP)
        weights[f"w1_{n}"] = np.ascontiguousarray(
            w1p.reshape(FCH, 128, HID).transpose(1, 0, 2)
        )
        w2 = np.asarray(W2n, np.float32).astype(MM2NP)
        weights[f"w2_{n}"] = np.ascontiguousarray(
            w2.reshape(JCH, 128, NCLS).transpose(1, 0, 2)
        )
        cpack[:, n * JCH : (n + 1) * JCH] = (
            np.asarray(b1n, np.float32).reshape(JCH, 128).T
        )
        cpack[:, 2 * JCH + n * NCLS : 2 * JCH + (n + 1) * NCLS] = (
            np.exp(np.asarray(b2n, np.float32))[None, :] / 16.0
        )
    weights["cpack"] = cpack

    in_maps = []
    for c in range(NCORES):
        m = dict(weights)
        for n in range(2):
            shard = xf[n, c * BC : (c + 1) * BC, :]  # [BC, F]
            m[f"xt{n}"] = pad_rows(
                np.ascontiguousarray(shard.T).astype(MM1NP), MM1NP
            )
        in_maps.append(m)

    nc = _get_nc()
    if _trace:
        _install_ntff_hook()
    res = run_bass_kernel_spmd(nc, in_maps, list(range(NCORES)), trace=_trace)
    pieces = [res.results[c]["out"] for c in range(NCORES)]
    full = np.concatenate(pieces, axis=0).astype(np.float32)
    if _trace:
        return full, res
    return full


# revision 33
# speedup vs baseline: 1.1919x; 1.1919x over previous
"""Trainium2 Bass kernel for NetworksPlusCircuit.

Two MLPs (784->512->10, relu, softmax) over B=65536 samples each, then
P(sum=s) = sum_{a+c=s} p0[a]*p1[c]  -> [B, 19].

Sharding: pure data parallel over the batch across 8 NeuronCores.
Host-side prep: shard + transpose x to [784, B/8] per core and cast to
fp16 so the device DMA streams are halved and contiguous; weights
replicated (cast to fp16 on host).

Device-side per core (BC = 8192 samples per net):
  per round (512 samples of one net):
    - HWDGE loads xT tile [128f, 7fc, 1024b] fp16, one DMA per tile
      (each DMA costs ~650ns ring time); the first tile is split fine so
      round 0 starts right after the framework preamble
    - mm1: hT[j, b] += W1[f, j].T @ xT[f, b]   (PSUM, 4 j-chunks x 7 f-chunks,
      16-row tail as 4 concurrent row-tiled strips)
    - bias+relu -> SBUF fp16 (ACT)
    - mm2: l[b, 10]  += hT[j, b].T @ W2[j, 10] one matmul per mm1 slot, so
      each mm2 LDWEIGHTS (~95ns) hides in weight-port slack under an mm1
      stream (216ns) instead of 16 LDWs serializing
    - exp (ACT, PSUM -> SBUF block buffer), softmax denominator deferred
  per block (3 pair-rounds; final block is 1 round so its conv tail is
  short):
    - scale by exp(b2)/16, Z sums, digit-sum convolution via a skewed fp16
      write + diagonal-strided reduce (3 DVE ops instead of 21), normalize,
      DMA out
"""

import sys
import types

import numpy as np
import ml_dtypes
import concourse.bass as bass
import concourse.bacc as bacc
import concourse.mybir as mybir
import concourse.tile as tile
from concourse.bass_utils import run_bass_kernel_spmd

F32 = mybir.dt.float32
MM1DT = mybir.dt.float16    # mm1 dtype (x, W1): fp16 = same PE speed as bf16,
MM1NP = np.float16          # 8x less rounding error (measured equal exec time)
MM2DT = mybir.dt.float16    # mm2 dtype (ht, W2)
MM2NP = np.float16

NCORES = 8
B = 65536
BC = B // NCORES            # samples per core per net
F = 784                     # input features
HID = 512
NCLS = 10
NSUM = 19
FCH_FULL = F // 128         # 6 full feature chunks
FT = F - FCH_FULL * 128     # 16 tail features
FCH = FCH_FULL + 1          # 7 chunks
JCH = HID // 128            # 4 hidden chunks
# The 16 tail features are replicated at 4 32-row strips (rows 768+32k..+16)
# so the 4 per-j-chunk tail matmuls sit in distinct PE row-groups and run
# concurrently (row tiling).
FPAD = FCH * 128  # padded rows: 6*128 full chunks + 4 strips * 32 = 896
RN = 512                    # samples per compute round
XR = 1024                   # samples per x-load tile
XRR = XR // RN              # compute rounds per x tile
NR = BC // RN               # 16 pair-rounds per core
# conv blocks in pair-rounds: uneven so the last block's conv (the only
# one that can't overlap matmuls of a later round) is small
BLK_ROUNDS = [3, 3, 3, 3, 3, 1]
assert sum(BLK_ROUNDS) == NR
GPR = RN // 128             # 128-sample groups per pair-round


def pad_rows(a, dtype):
    """[F, ...] -> [FPAD, ...] with tail rows replicated at 4 32-row strips."""
    out = np.zeros((FPAD,) + a.shape[1:], dtype=dtype)
    out[: FCH_FULL * 128] = a[: FCH_FULL * 128]
    for k in range(JCH):
        base = FCH_FULL * 128 + 32 * k
        out[base : base + FT] = a[FCH_FULL * 128 :]
    return out


def build_nc():
    nc = bacc.Bacc("TRN2", target_bir_lowering=False, debug=False, num_devices=NCORES)

    xt = [
        nc.dram_tensor(f"xt{n}", [FPAD, BC], MM1DT, kind="ExternalInput")
        for n in range(2)
    ]
    # weights arrive host-prepacked in their on-chip layouts so every const
    # DMA is one contiguous line per partition (cheap descriptor generation)
    w1 = [
        nc.dram_tensor(f"w1_{n}", [128, FCH, HID], MM1DT, kind="ExternalInput")
        for n in range(2)
    ]
    w2 = [
        nc.dram_tensor(f"w2_{n}", [128, JCH, NCLS], MM2DT, kind="ExternalInput")
        for n in range(2)
    ]
    # cpack[p] = [b1_0 (4), b1_1 (4), eb2_0 (10), eb2_1 (10)]
    cpk = nc.dram_tensor("cpack", [128, 28], F32, kind="ExternalInput")
    out = nc.dram_tensor("out", [BC, NSUM], F32, kind="ExternalOutput")

    with tile.TileContext(nc) as tc:
        with (
            tc.tile_pool(name="consts", bufs=1) as consts,
            tc.tile_pool(name="xt", bufs=8) as xt_pool,
            tc.tile_pool(name="ht", bufs=3) as ht_pool,
            tc.tile_pool(name="eblk", bufs=2) as e_pool,
            tc.tile_pool(name="conv", bufs=1) as conv_pool,
            tc.tile_pool(name="outp", bufs=3) as out_pool,
            tc.tile_pool(name="hpsum", bufs=7, space="PSUM") as h_pool,
            tc.tile_pool(name="lpsum", bufs=1, space="PSUM") as l_pool,
        ):
            # ---- constants (HWDGE on the ACT ring so the x-tile loads own
            # the sync ring from t=0) ----
            w1_sb = []
            w2_sb = []
            for n in range(2):
                w = consts.tile([128, FCH, HID], MM1DT, name=f"w1sb{n}", tag=f"w1sb{n}")
                w1_sb.append(w)
            cpack = consts.tile([128, 28], F32, name="cpack", tag="cpack")

            def w1_load(n):
                # three DMAs in first-use order (tail chunk 6, then the full
                # chunks in halves) so round-0 matmuls start as pieces land
                for c0, c1 in ((FCH_FULL, FCH), (0, 3), (3, FCH_FULL)):
                    nc.scalar.dma_start(
                        out=w1_sb[n][:, c0:c1, :], in_=w1[n].ap()[:, c0:c1, :]
                    )

            w1_load(0)
            nc.scalar.dma_start(out=cpack, in_=cpk.ap())
            w1_load(1)
            for n in range(2):
                w2t = consts.tile([128, JCH, NCLS], MM2DT, name=f"w2sb{n}", tag=f"w2sb{n}")
                nc.scalar.dma_start(out=w2t, in_=w2[n].ap())
                w2_sb.append(w2t)
            b1_sb = [cpack[:, 0:JCH], cpack[:, JCH : 2 * JCH]]
            # eb2 for both nets as [128, 2, NCLS]
            eb2_bn = cpack[:, 2 * JCH : 2 * JCH + 2 * NCLS].rearrange(
                "p (n c) -> p n c", n=2
            )
            # skew buffer for the digit-sum convolution: per group, row a
            # (stride 21) holds p0[a]*p1[c] at col a+c+1, so the diagonal
            # read (stride-20 view) sums exactly the a+c==s terms; all other
            # cells stay zero from this one-time memset (rows never move).
            MAXG = max(BLK_ROUNDS) * GPR
            # fp16 products: eb2 is pre-scaled by 1/16 on the host so the
            # p0*p1 products stay in fp16 range; the z normalization divides
            # the scaling back out (scale-invariant)
            skew = consts.tile([128, MAXG, 216], MM2DT, name="skew", tag="skew")
            nc.vector.memset(skew, 0.0)

            # ---- conv over a range of groups ----
            def emit_conv(ev, gc, row0):
                # ev: e slice [128, 2, gc, NCLS]; writes out rows row0..
                nc.vector.tensor_tensor(
                    out=ev,
                    in0=ev,
                    in1=eb2_bn[:, :, None, :].to_broadcast([128, 2, gc, NCLS]),
                    op=mybir.AluOpType.mult,
                )
                z = conv_pool.tile([128, 2, gc], F32, name="z", tag="z")
                nc.vector.reduce_sum(out=z, in_=ev, axis=mybir.AxisListType.X)
                rz = conv_pool.tile([128, gc], F32, name="rz", tag="rz")
                nc.vector.tensor_tensor(
                    out=rz, in0=z[:, 0, :], in1=z[:, 1, :], op=mybir.AluOpType.mult
                )
                nc.vector.reciprocal(out=rz, in_=rz)
                # skewed write: (g, a, c) -> col a+c+1 of the stride-21 row a
                wview = skew[:, 0:gc, 1:211].rearrange(
                    "p g (a u) -> p g a u", a=NCLS
                )[:, :, :, 0:NCLS]
                nc.vector.tensor_tensor(
                    out=wview,
                    in0=ev[:, 1, :, None, :].to_broadcast([128, gc, NCLS, NCLS]),
                    in1=ev[:, 0, :, :, None].to_broadcast([128, gc, NCLS, NCLS]),
                    op=mybir.AluOpType.mult,
                )
                # diagonal read: (g, s, a) at 20a+s+1 sums the a+c==s products
                rview = skew[:, 0:gc, 1:201].rearrange(
                    "p g (a u) -> p g a u", a=NCLS
                )[:, :, :, 0:NSUM].transpose([0, 1, 3, 2])
                acc = out_pool.tile([128, gc, NSUM], F32, name="acc", tag="acc")
                nc.vector.reduce_sum(out=acc, in_=rview, axis=mybir.AxisListType.X)
                nc.vector.tensor_tensor(
                    out=acc,
                    in0=acc,
                    in1=rz[:, :, None].to_broadcast([128, gc, NSUM]),
                    op=mybir.AluOpType.mult,
                )
                nc.scalar.dma_start(
                    out=out.ap()[row0 : row0 + gc * 128, :].rearrange(
                        "(g p) s -> p g s", p=128
                    ),
                    in_=acc,
                )

            # mm2 + exp for a finished round, emitted ONE SMALL MATMUL AT A
            # TIME between the next round's mm1 matmuls: each mm2 LDWEIGHTS
            # (~95ns) hides in the weight-port slack under an mm1 stream
            # (216ns - its own 97ns LDW), instead of 16 LDWs serializing.
            pend = {"v": None, "k": 16, "pl": None}

            def mm2_step():
                if pend["v"] is None or pend["k"] >= 16:
                    return
                pn, pht, pe_t, prr, last, pgpb, prow0 = pend["v"]
                k = pend["k"]
                bc, jc = divmod(k, JCH)
                if k == 0:
                    pend["pl"] = l_pool.tile(
                        [128, RN // 128, NCLS], F32, name="pl", tag="pl"
                    )
                nc.tensor.matmul(
                    out=pend["pl"][:, bc, :],
                    lhsT=pht[:, jc, bc * 128 : (bc + 1) * 128],
                    rhs=w2_sb[pn][:, jc, :],
                    start=(jc == 0),
                    stop=(jc == JCH - 1),
                    skip_group_check=True,
                )
                pend["k"] = k + 1
                if pend["k"] == 16:
                    nc.scalar.activation(
                        out=pe_t[:, pn, prr * GPR : (prr + 1) * GPR, :],
                        in_=pend["pl"][:, :, :],
                        func=mybir.ActivationFunctionType.Exp,
                    )
                    if pn == 1 and last:
                        emit_conv(pe_t, pgpb, prow0)

            # ---- main loop (mm2/exp software-pipelined one round behind) ----
            xtiles = {}
            r = 0
            row0 = 0
            for blk, rpb in enumerate(BLK_ROUNDS):
                gpb = rpb * GPR
                # exp values for this block: [128, net, group, cls]
                e_t = e_pool.tile([128, 2, gpb, NCLS], F32, name=f"eb{rpb}", tag=f"eb{rpb}")
                for rr in range(rpb):
                    # x tiles hold XR samples (several compute rounds):
                    # bigger DMA lines (XR*2B per partition) + fewer DMAs.
                    # net0's chunks are queued before net1's so the first
                    # rounds pipeline; the first tile loads in column halves
                    # so the first round's matmuls start ~2.5us in.
                    if r % XRR == 0:
                        for n in range(2):
                            xtiles[n] = xt_pool.tile(
                                [128, FCH, XR], MM1DT, name="xt", tag="xt"
                            )
                        if r == 0:
                            # first tile in consumption order: net0's first
                            # round finely chunked (tail chunk 6 first) so the
                            # first matmuls start ~1.5us after the rings open;
                            # later pieces coarser (each DMA costs ~650ns of
                            # ring time)
                            splits = [
                                (n, 0, FCH, b0, b1)
                                for b0, b1 in ((0, RN), (RN, XR))
                                for n in range(2)
                            ]
                        else:
                            # steady state: one DMA per tile (~650ns of ring
                            # time per DMA instruction, so fewer is better)
                            splits = [(n, 0, FCH, 0, XR) for n in range(2)]
                        for n, c0, c1, b0, b1 in splits:
                            xsrc = xt[n].ap()[:, r * RN + b0 : r * RN + b1]
                            nc.sync.dma_start(
                                out=xtiles[n][:, c0:c1, b0:b1],
                                in_=xsrc[c0 * 128 : c1 * 128, :].rearrange(
                                    "(c p) b -> p c b", p=128
                                ),
                            )
                    for n in range(2):
                        xtile = xtiles[n][:, :, (r % XRR) * RN : (r % XRR + 1) * RN]

                        ph = [
                            h_pool.tile([128, RN], F32, name="ph", tag="ph")
                            for i in range(JCH)
                        ]
                        # tail features first: 4 concurrent row-tiled K=16
                        # matmuls (start=True initializes every psum element)
                        for jc in range(JCH):
                            p0 = 32 * jc
                            nc.tensor.matmul(
                                out=ph[jc][:, :],
                                lhsT=w1_sb[n][
                                    p0 : p0 + FT, FCH_FULL, jc * 128 : (jc + 1) * 128
                                ],
                                rhs=xtile[p0 : p0 + FT, FCH_FULL, :],
                                start=True,
                                stop=False,
                                tile_position=(p0, 0),
                            )
                        ht = ht_pool.tile([128, JCH, RN], MM2DT, name="ht", tag="ht")
                        # jc-major: each j-chunk finishes early so its relu
                        # (all on ACT) drains the psum slot while the round runs
                        for jc in range(JCH):
                            for fc in range(FCH_FULL):
                                nc.tensor.matmul(
                                    out=ph[jc][:, :],
                                    lhsT=w1_sb[n][:, fc, jc * 128 : (jc + 1) * 128],
                                    rhs=xtile[:, fc, :],
                                    start=False,
                                    stop=(fc == FCH_FULL - 1),
                                )
                                mm2_step()
                            nc.scalar.activation(
                                out=ht[:, jc, :],
                                in_=ph[jc][:, :],
                                func=mybir.ActivationFunctionType.Relu,
                                bias=b1_sb[n][:, jc : jc + 1],
                                scale=1.0,
                            )
                        pend["v"] = (n, ht, e_t, rr, rr == rpb - 1, gpb, row0)
                        pend["k"] = 0
                    r += 1
                row0 += gpb * 128
            while pend["k"] < 16:
                mm2_step()

    nc.compile()
    return nc


_NC_CACHE = {}


def _get_nc():
    if "nc" not in _NC_CACHE:
        _NC_CACHE["nc"] = build_nc()
    return _NC_CACHE["nc"]


def _install_ntff_hook():
    """Shim antenv.axon_hooks (absent in this image) so trace=True can work."""
    try:
        import antenv

        if hasattr(antenv, "axon_hooks"):
            return
        from trn_agent_boot.trn_boot import _ntff_profile_via_ctypes

        mod = types.ModuleType("antenv.axon_hooks")
        holder = {"hook": _ntff_profile_via_ctypes("/opt/axon/libaxon_pjrt.so")}
        mod.set_axon_ntff_profile_hook = lambda h: holder.__setitem__("hook", h)
        mod.get_axon_ntff_profile_hook = lambda: holder["hook"]
        sys.modules["antenv.axon_hooks"] = mod
        antenv.axon_hooks = mod
    except Exception:
        pass


def kernel(x, W1_0, b1_0, W2_0, b2_0, W1_1, b1_1, W2_1, b2_1, _trace=False):
    x = np.asarray(x, dtype=np.float32)

    xf = x.reshape(2, B, F)
    weights = {}
    cpack = np.zeros((128, 28), np.float32)
    for n, (W1n, b1n, W2n, b2n) in enumerate(
        [(W1_0, b1_0, W2_0, b2_0), (W1_1, b1_1, W2_1, b2_1)]
    ):
        w1p = pad_rows(np.asarray(W1n, np.float32).astype(MM1NP), MM1NP)
        weights[f"w1_{n}"] = np.ascontiguousarray(
            w1p.reshape(FCH, 128, HID).transpose(1, 0, 2)
        )
        w2 = np.asarray(W2n, np.float32).astype(MM2NP)
        weights[f"w2_{n}"] = np.ascontiguousarray(
            w2.reshape(JCH, 128, NCLS).transpose(1, 0, 2)
        )
        cpack[:, n * JCH : (n + 1) * JCH] = (
            np.asarray(b1n, np.float32).reshape(JCH, 128).T
        )
        cpack[:, 2 * JCH + n * NCLS : 2 * JCH + (n + 1) * NCLS] = (
            np.exp(np.asarray(b2n, np.float32))[None, :] / 16.0
        )
    weights["cpack"] = cpack

    in_maps = []
    for c in range(NCORES):
        m = dict(weights)
        for n in range(2):
            shard = xf[n, c * BC : (c + 1) * BC, :]  # [BC, F]
            m[f"xt{n}"] = pad_rows(
                np.ascontiguousarray(shard.T).astype(MM1NP), MM1NP
            )
        in_maps.append(m)

    nc = _get_nc()
    if _trace:
        _install_ntff_hook()
    res = run_bass_kernel_spmd(nc, in_maps, list(range(NCORES)), trace=_trace)
    pieces = [res.results[c]["out"] for c in range(NCORES)]
    full = np.concatenate(pieces, axis=0).astype(np.float32)
    if _trace:
        return full, res
    return full


# revision 34
# speedup vs baseline: 1.2026x; 1.0090x over previous
"""Trainium2 Bass kernel for NetworksPlusCircuit.

Two MLPs (784->512->10, relu, softmax) over B=65536 samples each, then
P(sum=s) = sum_{a+c=s} p0[a]*p1[c]  -> [B, 19].

Sharding: pure data parallel over the batch across 8 NeuronCores.
Host-side prep: shard + transpose x to [784, B/8] per core and cast to
fp16 so the device DMA streams are halved and contiguous; weights
replicated (cast to fp16 on host).

Device-side per core (BC = 8192 samples per net):
  per round (512 samples of one net):
    - HWDGE loads xT tile [128f, 7fc, 1024b] fp16, one DMA per tile
      (each DMA costs ~650ns ring time); the first tile is split fine so
      round 0 starts right after the framework preamble
    - mm1: hT[j, b] += W1[f, j].T @ xT[f, b]   (PSUM, 4 j-chunks x 7 f-chunks,
      16-row tail as 4 concurrent row-tiled strips)
    - bias+relu -> SBUF fp16 (ACT)
    - mm2: l[b, 10]  += hT[j, b].T @ W2[j, 10] one matmul per mm1 slot, so
      each mm2 LDWEIGHTS (~95ns) hides in weight-port slack under an mm1
      stream (216ns) instead of 16 LDWs serializing
    - exp (ACT, PSUM -> SBUF block buffer), softmax denominator deferred
  per block (3 pair-rounds; final block is 1 round so its conv tail is
  short):
    - scale by exp(b2)/16, Z sums, digit-sum convolution via a skewed fp16
      write + diagonal-strided reduce (3 DVE ops instead of 21), normalize,
      DMA out
"""

import sys
import types

import numpy as np
import ml_dtypes
import concourse.bass as bass
import concourse.bacc as bacc
import concourse.mybir as mybir
import concourse.tile as tile
from concourse.bass_utils import run_bass_kernel_spmd

F32 = mybir.dt.float32
MM1DT = mybir.dt.float16    # mm1 dtype (x, W1): fp16 = same PE speed as bf16,
MM1NP = np.float16          # 8x less rounding error (measured equal exec time)
MM2DT = mybir.dt.float16    # mm2 dtype (ht, W2)
MM2NP = np.float16

NCORES = 8
B = 65536
BC = B // NCORES            # samples per core per net
F = 784                     # input features
HID = 512
NCLS = 10
NSUM = 19
FCH_FULL = F // 128         # 6 full feature chunks
FT = F - FCH_FULL * 128     # 16 tail features
FCH = FCH_FULL + 1          # 7 chunks
JCH = HID // 128            # 4 hidden chunks
# The 16 tail features are replicated at 4 32-row strips (rows 768+32k..+16)
# so the 4 per-j-chunk tail matmuls sit in distinct PE row-groups and run
# concurrently (row tiling).
FPAD = FCH * 128  # padded rows: 6*128 full chunks + 4 strips * 32 = 896
RN = 512                    # samples per compute round
XR = 1024                   # samples per x-load tile
XRR = XR // RN              # compute rounds per x tile
NR = BC // RN               # 16 pair-rounds per core
# conv blocks in pair-rounds: uneven so the last block's conv (the only
# one that can't overlap matmuls of a later round) is small
BLK_ROUNDS = [3, 3, 3, 3, 3, 1]
assert sum(BLK_ROUNDS) == NR
GPR = RN // 128             # 128-sample groups per pair-round


def pad_rows(a, dtype):
    """[F, ...] -> [FPAD, ...] with tail rows replicated at 4 32-row strips."""
    out = np.zeros((FPAD,) + a.shape[1:], dtype=dtype)
    out[: FCH_FULL * 128] = a[: FCH_FULL * 128]
    for k in range(JCH):
        base = FCH_FULL * 128 + 32 * k
        out[base : base + FT] = a[FCH_FULL * 128 :]
    return out


def build_nc():
    nc = bacc.Bacc("TRN2", target_bir_lowering=False, debug=False, num_devices=NCORES)

    xt = [
        nc.dram_tensor(f"xt{n}", [FPAD, BC], MM1DT, kind="ExternalInput")
        for n in range(2)
    ]
    # weights arrive host-prepacked in their on-chip layouts so every const
    # DMA is one contiguous line per partition (cheap descriptor generation)
    w1 = [
        nc.dram_tensor(f"w1_{n}", [128, FCH, HID], MM1DT, kind="ExternalInput")
        for n in range(2)
    ]
    w2 = [
        nc.dram_tensor(f"w2_{n}", [128, JCH, NCLS], MM2DT, kind="ExternalInput")
        for n in range(2)
    ]
    # cpack[p] = [b1_0 (4), b1_1 (4), eb2_0 (10), eb2_1 (10)]
    cpk = nc.dram_tensor("cpack", [128, 28], F32, kind="ExternalInput")
    out = nc.dram_tensor("out", [BC, NSUM], F32, kind="ExternalOutput")

    with tile.TileContext(nc) as tc:
        with (
            tc.tile_pool(name="consts", bufs=1) as consts,
            tc.tile_pool(name="xt", bufs=8) as xt_pool,
            tc.tile_pool(name="ht", bufs=3) as ht_pool,
            tc.tile_pool(name="eblk", bufs=2) as e_pool,
            tc.tile_pool(name="conv", bufs=1) as conv_pool,
            tc.tile_pool(name="outp", bufs=3) as out_pool,
            tc.tile_pool(name="hpsum", bufs=7, space="PSUM") as h_pool,
            tc.tile_pool(name="lpsum", bufs=1, space="PSUM") as l_pool,
        ):
            # ---- constants (HWDGE on the ACT ring so the x-tile loads own
            # the sync ring from t=0) ----
            w1_sb = []
            w2_sb = []
            for n in range(2):
                w = consts.tile([128, FCH, HID], MM1DT, name=f"w1sb{n}", tag=f"w1sb{n}")
                w1_sb.append(w)
            cpack = consts.tile([128, 28], F32, name="cpack", tag="cpack")

            def w1_load(n):
                # three DMAs in first-use order (tail chunk 6, then the full
                # chunks in halves) so round-0 matmuls start as pieces land
                for c0, c1 in ((FCH_FULL, FCH), (0, 3), (3, FCH_FULL)):
                    nc.scalar.dma_start(
                        out=w1_sb[n][:, c0:c1, :], in_=w1[n].ap()[:, c0:c1, :]
                    )

            w1_load(0)
            nc.scalar.dma_start(out=cpack, in_=cpk.ap())
            w1_load(1)
            for n in range(2):
                w2t = consts.tile([128, JCH, NCLS], MM2DT, name=f"w2sb{n}", tag=f"w2sb{n}")
                nc.scalar.dma_start(out=w2t, in_=w2[n].ap())
                w2_sb.append(w2t)
            b1_sb = [cpack[:, 0:JCH], cpack[:, JCH : 2 * JCH]]
            # eb2 for both nets as [128, 2, NCLS]
            eb2_bn = cpack[:, 2 * JCH : 2 * JCH + 2 * NCLS].rearrange(
                "p (n c) -> p n c", n=2
            )
            # skew buffer for the digit-sum convolution: per group, row a
            # (stride 21) holds p0[a]*p1[c] at col a+c+1, so the diagonal
            # read (stride-20 view) sums exactly the a+c==s terms; all other
            # cells stay zero from this one-time memset (rows never move).
            MAXG = max(BLK_ROUNDS) * GPR
            # fp16 products: eb2 is pre-scaled by 1/16 on the host so the
            # p0*p1 products stay in fp16 range; the z normalization divides
            # the scaling back out (scale-invariant)
            skew = consts.tile([128, MAXG, 216], MM2DT, name="skew", tag="skew")
            nc.vector.memset(skew, 0.0)

            # ---- conv over a range of groups ----
            def emit_conv(ev, gc, row0):
                # ev: e slice [128, 2, gc, NCLS]; writes out rows row0..
                nc.vector.tensor_tensor(
                    out=ev,
                    in0=ev,
                    in1=eb2_bn[:, :, None, :].to_broadcast([128, 2, gc, NCLS]),
                    op=mybir.AluOpType.mult,
                )
                z = conv_pool.tile([128, 2, gc], F32, name="z", tag="z")
                nc.vector.reduce_sum(out=z, in_=ev, axis=mybir.AxisListType.X)
                rz = conv_pool.tile([128, gc], F32, name="rz", tag="rz")
                nc.vector.tensor_tensor(
                    out=rz, in0=z[:, 0, :], in1=z[:, 1, :], op=mybir.AluOpType.mult
                )
                nc.vector.reciprocal(out=rz, in_=rz)
                # skewed write: (g, a, c) -> col a+c+1 of the stride-21 row a
                wview = skew[:, 0:gc, 1:211].rearrange(
                    "p g (a u) -> p g a u", a=NCLS
                )[:, :, :, 0:NCLS]
                nc.vector.tensor_tensor(
                    out=wview,
                    in0=ev[:, 1, :, None, :].to_broadcast([128, gc, NCLS, NCLS]),
                    in1=ev[:, 0, :, :, None].to_broadcast([128, gc, NCLS, NCLS]),
                    op=mybir.AluOpType.mult,
                )
                # diagonal read: (g, s, a) at 20a+s+1 sums the a+c==s products
                rview = skew[:, 0:gc, 1:201].rearrange(
                    "p g (a u) -> p g a u", a=NCLS
                )[:, :, :, 0:NSUM].transpose([0, 1, 3, 2])
                acc = out_pool.tile([128, gc, NSUM], F32, name="acc", tag="acc")
                nc.vector.reduce_sum(out=acc, in_=rview, axis=mybir.AxisListType.X)
                nc.vector.tensor_tensor(
                    out=acc,
                    in0=acc,
                    in1=rz[:, :, None].to_broadcast([128, gc, NSUM]),
                    op=mybir.AluOpType.mult,
                )
                nc.scalar.dma_start(
                    out=out.ap()[row0 : row0 + gc * 128, :].rearrange(
                        "(g p) s -> p g s", p=128
                    ),
                    in_=acc,
                )

            # mm2 + exp for a finished round, emitted ONE SMALL MATMUL AT A
            # TIME between the next round's mm1 matmuls: each mm2 LDWEIGHTS
            # (~95ns) hides in the weight-port slack under an mm1 stream
            # (216ns - its own 97ns LDW), instead of 16 LDWs serializing.
            pend = {"v": None, "k": 16, "pl": None}

            def mm2_step():
                if pend["v"] is None or pend["k"] >= 16:
                    return
                pn, pht, pe_t, prr, last, pgpb, prow0 = pend["v"]
                k = pend["k"]
                bc, jc = divmod(k, JCH)
                if k == 0:
                    pend["pl"] = l_pool.tile(
                        [128, RN // 128, NCLS], F32, name="pl", tag="pl"
                    )
                nc.tensor.matmul(
                    out=pend["pl"][:, bc, :],
                    lhsT=pht[:, jc, bc * 128 : (bc + 1) * 128],
                    rhs=w2_sb[pn][:, jc, :],
                    start=(jc == 0),
                    stop=(jc == JCH - 1),
                    skip_group_check=True,
                )
                pend["k"] = k + 1
                if pend["k"] == 16:
                    nc.scalar.activation(
                        out=pe_t[:, pn, prr * GPR : (prr + 1) * GPR, :],
                        in_=pend["pl"][:, :, :],
                        func=mybir.ActivationFunctionType.Exp,
                    )
                    if pn == 1 and last:
                        emit_conv(pe_t, pgpb, prow0)

            # ---- main loop (mm2/exp software-pipelined one round behind) ----
            xtiles = {}
            r = 0
            row0 = 0
            for blk, rpb in enumerate(BLK_ROUNDS):
                gpb = rpb * GPR
                # exp values for this block: [128, net, group, cls]
                e_t = e_pool.tile([128, 2, gpb, NCLS], F32, name=f"eb{rpb}", tag=f"eb{rpb}")
                for rr in range(rpb):
                    # x tiles hold XR samples (several compute rounds):
                    # bigger DMA lines (XR*2B per partition) + fewer DMAs.
                    # net0's chunks are queued before net1's so the first
                    # rounds pipeline; the first tile loads in column halves
                    # so the first round's matmuls start ~2.5us in.
                    if r % XRR == 0:
                        for n in range(2):
                            xtiles[n] = xt_pool.tile(
                                [128, FCH, XR], MM1DT, name="xt", tag="xt"
                            )
                        if r == 0:
                            # first tile in consumption order: net0's first
                            # round finely chunked (tail chunk 6 first) so the
                            # first matmuls start right after the rings open;
                            # later pieces coarser (each DMA costs ~650ns of
                            # ring time)
                            splits = [
                                (0, 6, 7, 0, RN),
                                (0, 0, 3, 0, RN),
                                (0, 3, 6, 0, RN),
                                (1, 6, 7, 0, RN),
                                (1, 0, 6, 0, RN),
                                (0, 0, FCH, RN, XR),
                                (1, 0, FCH, RN, XR),
                            ]
                        else:
                            # steady state: one DMA per tile (~650ns of ring
                            # time per DMA instruction, so fewer is better)
                            splits = [(n, 0, FCH, 0, XR) for n in range(2)]
                        for n, c0, c1, b0, b1 in splits:
                            xsrc = xt[n].ap()[:, r * RN + b0 : r * RN + b1]
                            nc.sync.dma_start(
                                out=xtiles[n][:, c0:c1, b0:b1],
                                in_=xsrc[c0 * 128 : c1 * 128, :].rearrange(
                                    "(c p) b -> p c b", p=128
                                ),
                            )
                    for n in range(2):
                        xtile = xtiles[n][:, :, (r % XRR) * RN : (r % XRR + 1) * RN]

                        ph = [
                            h_pool.tile([128, RN], F32, name="ph", tag="ph")
                            for i in range(JCH)
                        ]
                        # tail features first: 4 concurrent row-tiled K=16
                        # matmuls (start=True initializes every psum element)
                        for jc in range(JCH):
                            p0 = 32 * jc
                            nc.tensor.matmul(
                                out=ph[jc][:, :],
                                lhsT=w1_sb[n][
                                    p0 : p0 + FT, FCH_FULL, jc * 128 : (jc + 1) * 128
                                ],
                                rhs=xtile[p0 : p0 + FT, FCH_FULL, :],
                                start=True,
                                stop=False,
                                tile_position=(p0, 0),
                            )
                        ht = ht_pool.tile([128, JCH, RN], MM2DT, name="ht", tag="ht")
                        # jc-major: each j-chunk finishes early so its relu
                        # (all on ACT) drains the psum slot while the round runs
                        for jc in range(JCH):
                            for fc in range(FCH_FULL):
                                nc.tensor.matmul(
                                    out=ph[jc][:, :],
                                    lhsT=w1_sb[n][:, fc, jc * 128 : (jc + 1) * 128],
                                    rhs=xtile[:, fc, :],
                                    start=False,
                                    stop=(fc == FCH_FULL - 1),
                                )
                                mm2_step()
                            nc.scalar.activation(
                                out=ht[:, jc, :],
                                in_=ph[jc][:, :],
                                func=mybir.ActivationFunctionType.Relu,
                                bias=b1_sb[n][:, jc : jc + 1],
                                scale=1.0,
                            )
                        pend["v"] = (n, ht, e_t, rr, rr == rpb - 1, gpb, row0)
                        pend["k"] = 0
                    r += 1
                row0 += gpb * 128
            while pend["k"] < 16:
                mm2_step()

    nc.compile()
    return nc


_NC_CACHE = {}


def _get_nc():
    if "nc" not in _NC_CACHE:
        _NC_CACHE["nc"] = build_nc()
    return _NC_CACHE["nc"]


def _install_ntff_hook():
    """Shim antenv.axon_hooks (absent in this image) so trace=True can work."""
    try:
        import antenv

        if hasattr(antenv, "axon_hooks"):
            return
        from trn_agent_boot.trn_boot import _ntff_profile_via_ctypes

        mod = types.ModuleType("antenv.axon_hooks")
        holder = {"hook": _ntff_profile_via_ctypes("/opt/axon/libaxon_pjrt.so")}
        mod.set_axon_ntff_profile_hook = lambda h: holder.__setitem__("hook", h)
        mod.get_axon_ntff_profile_hook = lambda: holder["hook"]
        sys.modules["antenv.axon_hooks"] = mod
        antenv.axon_hooks = mod
    except Exception:
        pass


def kernel(x, W1_0, b1_0, W2_0, b2_0, W1_1, b1_1, W2_1, b2_1, _trace=False):
    x = np.asarray(x, dtype=np.float32)

    xf = x.reshape(2, B, F)
    weights = {}
    cpack = np.zeros((128, 28), np.float32)
    for n, (W1n, b1n, W2n, b2n) in enumerate(
        [(W1_0, b1_0, W2_0, b2_0), (W1_1, b1_1, W2_1, b2_1)]
    ):
        w1p = pad_rows(np.asarray(W1n, np.float32).astype(MM1NP), MM1NP)
        weights[f"w1_{n}"] = np.ascontiguousarray(
            w1p.reshape(FCH, 128, HID).transpose(1, 0, 2)
        )
        w2 = np.asarray(W2n, np.float32).astype(MM2NP)
        weights[f"w2_{n}"] = np.ascontiguousarray(
            w2.reshape(JCH, 128, NCLS).transpose(1, 0, 2)
        )
        cpack[:, n * JCH : (n + 1) * JCH] = (
            np.asarray(b1n, np.float32).reshape(JCH, 128).T
        )
        cpack[:, 2 * JCH + n * NCLS : 2 * JCH + (n + 1) * NCLS] = (
            np.exp(np.asarray(b2n, np.float32))[None, :] / 16.0
        )
    weights["cpack"] = cpack

    in_maps = []
    for c in range(NCORES):
        m = dict(weights)
        for n in range(2):
            shard = xf[n, c * BC : (c + 1) * BC, :]  # [BC, F]
            m[f"xt{n}"] = pad_rows(
                np.ascontiguousarray(shard.T).astype(MM1NP), MM1NP
            )
        in_maps.append(m)

    nc = _get_nc()
    if _trace:
        _install_ntff_hook()
    res = run_bass_kernel_spmd(nc, in_maps, list(range(NCORES)), trace=_trace)
    pieces = [res.results[c]["out"] for c in range(NCORES)]
    full = np.concatenate(pieces, axis=0).astype(np.float32)
    if _trace:
        return full, res
    return full
